# revision 17
# baseline (speedup 1.0000x reference)
"""Trainium2 Bass kernel for nn_BasicBlock (FBS-masked ternary conv + BN + LeakyReLU).

Sharding: data-parallel over batch. B=32 -> 4 samples per core on 8 cores.

Key ideas vs a straightforward f32r implementation:
  - The ternary weights take only 3 values {pos, 0, neg}.  We pick fp8-e4m3-
    exact u, v with v/u ~ r = neg/pos (best ratio pair, ~0.3% error) and run
    the conv entirely in fp8 with MatmulPerfMode.DoubleRow (0.5 cycles/row,
    256-wide contraction): both ci tiles are packed into the two DoubleRow
    halves.  x is split x = hi + lo (e4m3 each); hi and lo passes accumulate
    into the same PSUM banks, recovering ~bf16 accuracy (~0.3% overall).
  - Conv output y' = (u/pos) * y_true.  BN absorbs the scale except through
    eps: eps' = eps * (u/pos)^2 (host immediate).
  - Quadrant decomposition for the stride-2 K=4 conv: 4 parity quadrants,
    33-wide rows (one shared zero pad col) so every tap is a single
    contiguous flat run per PSUM bank (rank-3 APs for DoubleRow).
  - y stays in SBUF as bf16 (no DRAM round trip); the FBS mask is folded
    into the per-(sample, channel-tile) epilogue scale; BN batch stats go
    through one AllGather (cheaper than AllReduce in practice) + local sum.
  - Top-k threshold (k=410 of 512) is exact: count matrix via compare +
    ones-matmul, thr = min{s_j : #{s_i > s_j} <= 409}.
"""

import numpy as np
import ml_dtypes

import concourse.bass as bass
import concourse.mybir as mybir
import concourse.tile as tile
from concourse.bass_utils import run_bass_kernel_spmd
from concourse.masks import make_identity

F32 = mybir.dt.float32
BF16 = mybir.dt.bfloat16
F8 = mybir.dt.float8e4
AF = mybir.ActivationFunctionType
ALU = mybir.AluOpType
AX = mybir.AxisListType
DR = mybir.MatmulPerfMode.DoubleRow

N_CORES = 8
B, CIN, H, W = 32, 256, 64, 64
COUT, KK = 512, 4
OH, OW = 32, 32
NB = B // N_CORES          # samples per core = 4
NT = CIN // 128            # ci tiles = 2
NCOT = COUT // 128         # co tiles = 4
CR_KEEP = 409.5            # count <= 409  <->  count < 409.5
BN_EPS = 1e-5
NEG_SLOPE = 0.2
THRESH_FACTOR = 0.05
NSP = OH * OW              # 1024 spatial positions per sample
W33 = OW + 1               # quad row stride; col 32 is a shared zero pad

MAX_WAITS = 1              # this walrus build allows 1 sync wait per instruction

# kh -> (row parity ph, row shift dj): x row 2*oh + kh - 1 = 2*(oh+dj) + ph
PAR = {0: (1, -1), 1: (0, 0), 2: (1, 0), 3: (0, 1)}
KHW_ORDER = [(1, 1)] + [(kh, kw) for kh in range(KK) for kw in range(KK)
                        if (kh, kw) != (1, 1)]
BIG = 1.0e30


def _split_waits(nc, max_waits=MAX_WAITS):
    """Split per-instruction sem waits exceeding max_waits into preceding
    same-engine InstNoOp carriers (engines execute their queue in order)."""
    for f in nc.m.functions:
        for bb in f.blocks:
            new_list = []
            changed = False
            for ins in bb.instructions:
                si = ins.sync_info
                if si is not None and si.on_wait and len(si.on_wait) > max_waits:
                    waits = list(si.on_wait)
                    carry = waits[: len(waits) - max_waits]
                    keep = waits[len(waits) - max_waits:]
                    k = 0
                    while carry:
                        chunk, carry = carry[:max_waits], carry[max_waits:]
                        new_list.append(
                            mybir.InstNoOp(
                                name=f"{ins.name}_ws{k}",
                                engine=ins.engine,
                                bass_nofuse=True,
                                sync_info=mybir.SyncInfo(on_wait=chunk, on_update=[]),
                            )
                        )
                        k += 1
                    ins.sync_info = mybir.SyncInfo(
                        on_wait=keep, on_update=list(si.on_update)
                    )
                    changed = True
                new_list.append(ins)
            if changed:
                bb.instructions = new_list


def best_fp8_pair(r):
    """e4m3-exact (u, v) minimizing |v/u - r|/|r|."""
    best = None
    for m in range(8, 16):
        for k in range(-3, 4):
            u = m * (2.0 ** k) / 8.0
            v = float(np.float32(u * r).astype(ml_dtypes.float8_e4m3fn)
                      .astype(np.float32))
            if v == 0.0 or abs(v) > 448:
                continue
            err = abs(v / u - r) / abs(r)
            if best is None or err < best[0]:
                best = (err, u, v)
    return best[1], best[2]


def build_kernel(u_imm, v_imm, eps_imm, debug=False, sim_compat=False):
    nc = bass.Bass()

    xs = nc.dram_tensor("xs", [NB, CIN, H, W], F32, kind="ExternalInput")
    wt = nc.dram_tensor("wt", [COUT, CIN, KK, KK], F32, kind="ExternalInput")
    salw = nc.dram_tensor("salw", [COUT, CIN], F32, kind="ExternalInput")
    salb = nc.dram_tensor("salb", [COUT], F32, kind="ExternalInput")
    gam = nc.dram_tensor("gam", [COUT], F32, kind="ExternalInput")
    bet = nc.dram_tensor("bet", [COUT], F32, kind="ExternalInput")
    out = nc.dram_tensor("out", [NB, COUT, OH, OW], F32, kind="ExternalOutput")

    cc_out = nc.dram_tensor("cc_out", [2, N_CORES, 128, 4], F32,
                            addr_space="Shared")

    if debug:
        dbg_sal = nc.dram_tensor("dbg_sal", [NB, COUT], F32, kind="ExternalOutput")
        dbg_thr = nc.dram_tensor("dbg_thr", [NB, 1], F32, kind="ExternalOutput")
        dbg_mask = nc.dram_tensor("dbg_mask", [NB, COUT], F32, kind="ExternalOutput")
        dbg_y = nc.dram_tensor("dbg_y", [NB, NCOT, 128, NSP], F32,
                               kind="ExternalOutput")
        dbg_st = nc.dram_tensor("dbg_st", [128, 2 * NCOT], F32,
                                kind="ExternalOutput")
        dbg_wq = nc.dram_tensor("dbg_wq", [128, 16 * NT * COUT], F32,
                                kind="ExternalOutput")

    with tile.TileContext(nc) as tc:
        with (
            tc.tile_pool(name="persist", bufs=1) as pp,
            tc.tile_pool(name="wsq", bufs=16) as wqp,
            tc.tile_pool(name="xst", bufs=3) as xsp,
            tc.tile_pool(name="gst", bufs=2) as gsp,
            tc.tile_pool(name="small", bufs=4) as smp,
            tc.tile_pool(name="otp", bufs=2) as otp,
            tc.tile_pool(name="psum", bufs=1, space="PSUM") as psp,
            tc.tile_pool(name="dram", bufs=1, space="DRAM") as dp,
        ):
            # ---------- constants ----------
            ident = pp.tile([128, 128], F32, name="ident")
            make_identity(nc, ident)
            ident8 = pp.tile([128, 128], F8, name="ident8")
            nc.gpsimd.tensor_copy(ident8, ident)
            onesk = pp.tile([128, NB], BF16, name="onesk")
            nc.vector.memset(onesk, 1.0)
            ones1 = pp.tile([1, 128], F32, name="ones1")
            nc.vector.memset(ones1, 1.0)
            ebs = []
            ehot = []
            for b in range(NB):
                eb = pp.tile([NB, 128], F32, name=f"eb{b}")
                nc.gpsimd.memset(eb, 0.0)
                # eb[x, y] = (x == b) ? 1 : 0
                nc.gpsimd.affine_select(
                    out=eb, in_=eb, compare_op=ALU.not_equal, fill=1.0,
                    base=-b, pattern=[[0, 128]], channel_multiplier=1)
                ebs.append(eb)
                eh = pp.tile([128, NB], BF16, name=f"eh{b}")
                nc.gpsimd.memset(eh, 0.0)
                # eh[x, y] = (y == b) ? 1 : 0
                nc.gpsimd.affine_select(
                    out=eh, in_=eh, compare_op=ALU.not_equal, fill=1.0,
                    base=-b, pattern=[[1, NB]], channel_multiplier=0)
                ehot.append(eh)

            def col128(dram_vec, nm):  # [512] dram -> [128,4] sbuf
                t_ = pp.tile([128, NCOT], F32, name=nm)
                ap = bass.AP(tensor=dram_vec, offset=0, ap=[[1, 128], [128, NCOT]])
                nc.sync.dma_start(out=t_, in_=ap)
                return t_

            # ---------- DMA: w (natural layout), saliency consts, then x ----
            wst = []  # 16 subchunks [128co, 1024 = 64ci*16khw]; (c, q)
            mx = pp.tile([128, 16], F32, name="mx")
            for c in range(NCOT):
                for q in range(4):
                    ws_ = wqp.tile([128, 1024], F32, name=f"ws{c}{q}",
                                   tag="wsq", padded_shape=[128, 1056])
                    nc.sync.dma_start(
                        out=ws_,
                        in_=wt[c * 128:(c + 1) * 128,
                               q * 64:(q + 1) * 64, :, :].rearrange(
                                   "co ci kh kw -> co (ci kh kw)"))
                    wst.append(ws_)
                    nc.vector.tensor_reduce(
                        mx[:, c * 4 + q: c * 4 + q + 1], ws_, axis=AX.X,
                        op=ALU.max, apply_absolute_value=True)

            salb_t = col128(salb, "salb_t")
            gam_t = col128(gam, "gam_t")
            bet_t = col128(bet, "bet_t")
            swn = []
            for c in range(NCOT):
                sw_ = pp.tile([128, CIN], F32, name=f"swn{c}")
                nc.sync.dma_start(out=sw_, in_=salw[c * 128:(c + 1) * 128, :])
                swn.append(sw_)

            # x half-sample stages; order b-major so quads complete in order
            stg = {}
            for b in range(NB):
                for t in range(NT):
                    for hh in range(2):
                        s_ = xsp.tile([128, 32, W], F32, name=f"x{b}{t}{hh}",
                                      tag="x")
                        nc.sync.dma_start(
                            out=s_,
                            in_=xs[b, t * 128:(t + 1) * 128,
                                   hh * 32:(hh + 1) * 32, :])
                        stg[(b, t, hh)] = s_

            # ---------- global max |w| -> tcol/ntcol ----------
            mxr = pp.tile([128, 1], F32, name="mxr")
            nc.vector.tensor_reduce(mxr, mx, axis=AX.X, op=ALU.max)
            ps_s = psp.tile([128, 512], F32, name="ps_g", tag="small", bufs=2)
            nc.tensor.transpose(ps_s[0:1, 0:128], mxr, ident)
            gmaxrow = pp.tile([1, 128], F32, name="gmaxrow")
            nc.scalar.copy(gmaxrow, ps_s[0:1, 0:128])
            gmax = pp.tile([1, 1], F32, name="gmax")
            nc.vector.tensor_reduce(gmax, gmaxrow, axis=AX.X, op=ALU.max)
            ps_b = psp.tile([128, 512], F32, name="ps_b", tag="small", bufs=2)
            nc.tensor.matmul(ps_b[:, 0:1], ones1, gmax, start=True, stop=True)
            tcol = pp.tile([128, 1], F32, name="tcol")
            nc.scalar.activation(tcol, ps_b[:, 0:1], AF.Copy, bias=0.0,
                                 scale=float(THRESH_FACTOR))
            ntcol = pp.tile([128, 1], F32, name="ntcol")
            nc.scalar.activation(ntcol, ps_b[:, 0:1], AF.Copy, bias=0.0,
                                 scale=-float(THRESH_FACTOR))

            # ---------- ternarize + transpose into DoubleRow lhsT layout ----
            # wq[p=ci%128, khw, t, co] fp8; lhsT slice [128, 2, 128] per (khw, cot)
            wq = pp.tile([128, 16, NT, COUT], F8, name="wq")
            for c in range(NCOT):          # co chunk (c-major: conv needs c0 first)
                for t in range(NT):
                    gt = gsp.tile([128, 2048], F8, name=f"gt{c}{t}", tag="gt")
                    for qq in range(2):    # the two 64-ci subchunks of this t
                        q = t * 2 + qq
                        wsrc = wst[c * 4 + q]
                        g1 = gsp.tile([128, 1024], F8, name=f"g1_{c}{q}", tag="g1")
                        nc.vector.tensor_scalar(g1, wsrc, tcol[:, :],
                                                float(u_imm),
                                                op0=ALU.is_gt, op1=ALU.mult)
                        g2 = gsp.tile([128, 1024], F8, name=f"g2_{c}{q}", tag="g2")
                        if q % 2 == 0:
                            eng2, enga = nc.gpsimd, nc.vector
                        else:
                            eng2, enga = nc.vector, nc.gpsimd
                        eng2.tensor_scalar(g2, wsrc, ntcol[:, :], float(v_imm),
                                           op0=ALU.is_lt, op1=ALU.mult)
                        enga.tensor_tensor(out=gt[:, qq * 1024:(qq + 1) * 1024],
                                           in0=g1, in1=g2, op=ALU.add)
                    # transpose [co128, ci128] blocks per khw into psum
                    ps_t = psp.tile([128, 2048], F8, name=f"pt{c}{t}",
                                    tag="small", bufs=2)
                    for khw in range(16):
                        src = bass.AP(tensor=gt.tensor, offset=gt.offset + khw,
                                      ap=[gt.ap[0], [16, 128]])
                        nc.tensor.transpose(ps_t[:, khw * 128:(khw + 1) * 128],
                                            src, ident8)
                    dst = wq[:, :, t, c * 128:(c + 1) * 128]
                    engc = nc.scalar if (c % 2 == 0) else nc.gpsimd
                    if engc is nc.scalar:
                        nc.scalar.copy(dst, ps_t)
                    else:
                        nc.gpsimd.tensor_copy(dst, ps_t)

            if debug:
                wq32 = pp.tile([128, 16 * NT * COUT], F32, name="wq32")
                nc.vector.tensor_copy(
                    wq32, wq.rearrange("p a t c -> p (a t c)"))
                nc.sync.dma_start(out=dbg_wq[:, :], in_=wq32)

            # ---------- quad tiles (reuse w-chunk slots) + pad zeroing ----
            # Q[hl][b][ph] = [128, 2(pw), 2(t), 32(oh), 33]; col 32 is zero pad
            # creation order b-major so Q(b) lands on the slot of w chunk c=b,
            # whose ternarize completes before sample b's data arrives.
            Q = [[[None for ph in range(2)] for b in range(NB)] for hl in range(2)]
            for b in range(NB):
                for hl in range(2):
                    for ph in range(2):
                        qt_ = wqp.tile([128, 2, NT, OH, W33], F8,
                                       name=f"q{hl}{b}{ph}", tag="wsq")
                        nc.gpsimd.memset(qt_[:, :, :, :, OW:W33], 0.0)
                        Q[hl][b][ph] = qt_

            # ---------- salw transposes (PE, pre-conv) ----------
            salwT = [pp.tile([128, COUT], F32, name=f"swT{t}") for t in range(NT)]
            for c in range(NCOT):
                for t in range(NT):
                    ps_w = psp.tile([128, 512], F32, name=f"pw{c}{t}",
                                    tag="small", bufs=2)
                    nc.tensor.transpose(ps_w[:, 0:128],
                                        swn[c][:, t * 128:(t + 1) * 128], ident)
                    nc.scalar.copy(salwT[t][:, c * 128:(c + 1) * 128],
                                   ps_w[:, 0:128])

            # ---------- x quantize + interleave + |x| sums ----------
            subT = [pp.tile([128, NB], F32, name=f"subT{t}") for t in range(NT)]
            subh = smp.tile([128, NT * NB * 2], F32, name="subh", tag="subh",
                            bufs=1)

            def interleave(b, t, hh):
                s_ = stg[(b, t, hh)]
                # |x| partial sums on ACT (scratch) for hh=0, DVE reduce hh=1
                col = (t * NB + b) * 2 + hh
                if hh == 0:
                    scr = xsp.tile([128, 32 * W], F32, name=f"sc{b}{t}",
                                   tag="scr", bufs=2)
                    nc.scalar.activation(scr, s_.rearrange("p a b -> p (a b)"),
                                         AF.Abs, bias=0.0, scale=1.0,
                                         accum_out=subh[:, col:col + 1])
                else:
                    nc.vector.tensor_reduce(subh[:, col:col + 1], s_,
                                            axis=AX.XY, op=ALU.add,
                                            apply_absolute_value=True)
                for ph in range(2):
                    # in: stg rows ph::2 viewed (pw, r, w); out: Q[., pw, t, hh16+r, w]
                    src = bass.AP(tensor=s_.tensor, offset=s_.offset + ph * W,
                                  ap=[s_.ap[0], [1, 2], [2 * W, 16], [2, OW]])
                    qhi = Q[0][b][ph][:, :, t, hh * 16:(hh + 1) * 16, 0:OW]
                    nc.scalar.copy(qhi, src)
                    qlo = Q[1][b][ph][:, :, t, hh * 16:(hh + 1) * 16, 0:OW]
                    nc.vector.tensor_tensor(out=qlo, in0=src, in1=qhi,
                                            op=ALU.subtract)

            for b in (0, 1):
                for t in range(NT):
                    for hh in range(2):
                        interleave(b, t, hh)

            # ---------- conv ----------
            y = [[pp.tile([128, NSP], BF16, name=f"y{b}{c}") for c in range(NCOT)]
                 for b in range(NB)]
            s1 = [pp.tile([128, 2 * NB], F32, name=f"s1_{c}") for c in range(NCOT)]
            s2 = [pp.tile([128, 2 * NB], F32, name=f"s2_{c}") for c in range(NCOT)]

            def conv_group(b, cot):
                halves = [psp.tile([128, 1024], F32, name=f"bk{b}{cot}{h}",
                                   tag="conv", bufs=3) for h in range(2)]
                n_mm = 0
                total = 2 * 16 * 8
                for hl in range(2):
                    for kh, kw in KHW_ORDER:
                        ph, dj = PAR[kh]
                        pw, di = PAR[kw]
                        lhsT = wq[:, kh * KK + kw, :, cot * 128:(cot + 1) * 128]
                        qt = Q[hl][b][ph]
                        for h in range(2):
                            for j in range(2):
                                r0 = 16 * h + 8 * j
                                rows_lo = max(r0, -dj)
                                rows_hi = min(r0 + 7, OH - 1 - dj)
                                d0 = 512 * j + (rows_lo - r0) * W33
                                s0 = pw * (NT * OH * W33) + \
                                    (rows_lo + dj) * W33 + di
                                L = (rows_hi - rows_lo) * W33 + W33
                                if s0 < pw * (NT * OH * W33):
                                    d0 += 1
                                    s0 += 1
                                    L -= 1
                                rhs = bass.AP(
                                    tensor=qt.tensor, offset=qt.offset + s0,
                                    ap=[qt.ap[0], [OH * W33, NT], [1, L]])
                                nc.tensor.matmul(
                                    halves[h][:, d0:d0 + L], lhsT, rhs,
                                    start=(n_mm < 4), stop=(n_mm >= total - 4),
                                    perf_mode=DR, skip_group_check=True)
                                n_mm += 1
                for h in range(2):
                    src = bass.AP(tensor=halves[h].tensor,
                                  offset=halves[h].offset,
                                  ap=[halves[h].ap[0], [512, 2], [W33, 8],
                                      [1, OW]])
                    slot = b * 2 + h
                    nc.scalar.activation(
                        y[b][cot][:, h * 512:(h + 1) * 512], src, AF.Copy,
                        bias=0.0, scale=1.0,
                        accum_out=s1[cot][:, slot:slot + 1])
                    sq = smp.tile([128, 512], F32, name=f"sq{b}{cot}{h}",
                                  tag="sq", bufs=2)
                    nc.vector.tensor_tensor_reduce(
                        out=sq, in0=y[b][cot][:, h * 512:(h + 1) * 512],
                        in1=y[b][cot][:, h * 512:(h + 1) * 512],
                        scale=1.0, scalar=0.0, op0=ALU.mult, op1=ALU.add,
                        accum_out=s2[cot][:, slot:slot + 1])

            for cot in range(NCOT):
                conv_group(0, cot)
                if cot == 0:
                    for t in range(NT):
                        for hh in range(2):
                            interleave(2, t, hh)
            for cot in range(NCOT):
                conv_group(1, cot)
                if cot == 0:
                    for t in range(NT):
                        for hh in range(2):
                            interleave(3, t, hh)

            # ---------- saliency + top-k (emitted mid-conv; deps ready) ----
            subm = [pp.tile([128, NB], F32, name=f"subm{t}") for t in range(NT)]
            for t in range(NT):
                sview = bass.AP(tensor=subh.tensor,
                                offset=subh.offset + t * NB * 2,
                                ap=[subh.ap[0], [2, NB], [1, 2]])
                nc.vector.tensor_reduce(subT[t], sview, axis=AX.X, op=ALU.add)
                nc.vector.tensor_scalar(subm[t], subT[t], 1.0 / (H * W), None,
                                        op0=ALU.mult)
            sal_cb = []
            for cot in range(NCOT):
                ps_sal = psp.tile([128, 512], F32, name=f"psal{cot}",
                                  tag="small", bufs=2)
                for t in range(NT):
                    nc.tensor.matmul(ps_sal[:, 0:NB],
                                     salwT[t][:, cot * 128:(cot + 1) * 128],
                                     subm[t], start=(t == 0), stop=(t == NT - 1))
                sc = pp.tile([128, NB], F32, name=f"salcb{cot}")
                nc.scalar.activation(sc, ps_sal[:, 0:NB], AF.Abs,
                                     bias=salb_t[:, cot:cot + 1], scale=1.0)
                sal_cb.append(sc)
            salT = pp.tile([NB, COUT], F32, name="salT")
            for cot in range(NCOT):
                ps_st = psp.tile([128, 512], F32, name=f"pst{cot}",
                                 tag="small", bufs=2)
                nc.tensor.transpose(ps_st[0:NB, 0:128], sal_cb[cot], ident)
                nc.scalar.copy(salT[:, cot * 128:(cot + 1) * 128],
                               ps_st[0:NB, 0:128])
            if debug:
                nc.sync.dma_start(out=dbg_sal[:, :], in_=salT)

            # counts: CtAll[b, j] = #{i: sal[b,i] > sal[b,j]}; one-hot lhsT
            # routes sample b's counts to psum row b, accumulated across b.
            CtAll = pp.tile([NB, COUT], F32, name="CtAll")
            ps_c = psp.tile([128, 512], F32, name="pc", tag="small", bufs=2)
            for b in range(NB):
                ps_bc = psp.tile([128, 512], F32, name=f"pbc{b}",
                                 tag="small", bufs=2)
                nc.tensor.matmul(ps_bc[:, 0:COUT], ebs[b], salT,
                                 start=True, stop=True)
                bc = smp.tile([128, COUT], F32, name=f"bc{b}", tag="bc", bufs=2)
                nc.scalar.copy(bc, ps_bc[:, 0:COUT])
                for cot in range(NCOT):
                    cmp = smp.tile([128, COUT], BF16, name=f"cmp{b}{cot}",
                                   tag="cmp", bufs=2)
                    nc.vector.tensor_scalar(cmp, bc, sal_cb[cot][:, b:b + 1],
                                            None, op0=ALU.is_lt)
                    nc.tensor.matmul(ps_c[0:NB, :], ehot[b], cmp,
                                     start=(b == 0 and cot == 0),
                                     stop=(b == NB - 1 and cot == NCOT - 1),
                                     skip_group_check=True)
            nc.scalar.copy(CtAll, ps_c[0:NB, :])

            # thr[b] = min{sal : count <= 409}, all 4 samples in one [4, 512]
            m01 = smp.tile([NB, COUT], F32, name="m01", tag="tk", bufs=3)
            nc.vector.tensor_scalar(m01, CtAll, CR_KEEP, None, op0=ALU.is_lt)
            t2 = smp.tile([NB, COUT], F32, name="t2", tag="tk", bufs=3)
            nc.vector.tensor_scalar(t2, m01, -BIG, BIG, op0=ALU.mult,
                                    op1=ALU.add)
            t3 = smp.tile([NB, COUT], F32, name="t3", tag="tk", bufs=3)
            nc.vector.tensor_tensor(out=t3, in0=m01, in1=salT, op=ALU.mult)
            sel = smp.tile([NB, COUT], F32, name="sel", tag="tk", bufs=3)
            nc.vector.tensor_tensor(out=sel, in0=t3, in1=t2, op=ALU.add)
            thrc = pp.tile([NB, 1], F32, name="thrc")
            nc.vector.tensor_reduce(thrc, sel, axis=AX.X, op=ALU.min)
            if debug:
                nc.sync.dma_start(out=dbg_thr[:, :], in_=thrc)
            gtm = smp.tile([NB, COUT], F32, name="gtm", tag="tk", bufs=3)
            nc.vector.tensor_scalar(gtm, salT, thrc[:, :], None, op0=ALU.is_gt)
            maskT = pp.tile([NB, COUT], F32, name="maskT")
            nc.vector.tensor_tensor(out=maskT, in0=gtm, in1=salT, op=ALU.mult)
            if debug:
                nc.sync.dma_start(out=dbg_mask[:, :], in_=maskT)
            mask_cb, msq_cb = [], []
            for cot in range(NCOT):
                ps_m = psp.tile([128, 512], F32, name=f"pm{cot}",
                                tag="small", bufs=2)
                nc.tensor.transpose(ps_m[0:128, 0:NB],
                                    maskT[:, cot * 128:(cot + 1) * 128],
                                    ident[0:NB, 0:NB])
                mc = pp.tile([128, NB], F32, name=f"mcb{cot}")
                nc.scalar.copy(mc, ps_m[0:128, 0:NB])
                mask_cb.append(mc)
                mq = pp.tile([128, NB], F32, name=f"msq{cot}")
                nc.gpsimd.tensor_tensor(out=mq, in0=mc, in1=mc, op=ALU.mult)
                msq_cb.append(mq)

            # ---------- per-cot: stats + AllGather + BN + epilogue ----------
            inv_n = 1.0 / float(B * NSP)
            epst = pp.tile([128, 1], F32, name="epst")
            nc.vector.memset(epst, float(eps_imm))

            css = {}

            def cot_stats(cot):
                s1b = pp.tile([128, NB], F32, name=f"s1b{cot}")
                a0 = bass.AP(tensor=s1[cot].tensor, offset=s1[cot].offset,
                             ap=[s1[cot].ap[0], [2, NB], [1, 2]])
                nc.vector.tensor_reduce(s1b, a0, axis=AX.X, op=ALU.add)
                s2b = pp.tile([128, NB], F32, name=f"s2b{cot}")
                a1 = bass.AP(tensor=s2[cot].tensor, offset=s2[cot].offset,
                             ap=[s2[cot].ap[0], [2, NB], [1, 2]])
                nc.vector.tensor_reduce(s2b, a1, axis=AX.X, op=ALU.add)
                grp, sl = cot // 2, cot % 2
                if sl == 0:
                    css[grp] = pp.tile([128, 4], F32, name=f"cs{grp}")
                cs = css[grp]
                w1 = pp.tile([128, NB], F32, name=f"w1{cot}")
                nc.vector.tensor_tensor(out=w1, in0=s1b, in1=mask_cb[cot],
                                        op=ALU.mult)
                nc.vector.tensor_reduce(cs[:, 2 * sl:2 * sl + 1], w1,
                                        axis=AX.X, op=ALU.add)
                w2 = pp.tile([128, NB], F32, name=f"w2{cot}")
                nc.vector.tensor_tensor(out=w2, in0=s2b, in1=msq_cb[cot],
                                        op=ALU.mult)
                nc.vector.tensor_reduce(cs[:, 2 * sl + 1:2 * sl + 2], w2,
                                        axis=AX.X, op=ALU.add)

            sts = {}

            def grp_comm(grp):
                cc_in = dp.tile([128, 4], F32, name=f"cc_in{grp}")
                nc.sync.dma_start(out=cc_in, in_=css[grp])
                nc.gpsimd.collective_compute(
                    "AllGather", ALU.bypass,
                    replica_groups=[list(range(N_CORES))],
                    ins=[cc_in[:, :]], outs=[cc_out[grp, :, :, :]])
                gth = pp.tile([128, N_CORES, 4], F32, name=f"gth{grp}")
                nc.sync.dma_start(
                    out=gth,
                    in_=bass.AP(tensor=cc_out, offset=grp * N_CORES * 128 * 4,
                                ap=[[4, 128], [512, N_CORES], [1, 4]]))
                st = pp.tile([128, 4], F32, name=f"stt{grp}")
                nc.vector.tensor_reduce(
                    st, bass.AP(tensor=gth.tensor, offset=gth.offset,
                                ap=[gth.ap[0], [1, 4], [4, N_CORES]]),
                    axis=AX.X, op=ALU.add)
                sts[grp] = st
                if debug:
                    nc.sync.dma_start(out=dbg_st[:, 4 * grp:4 * grp + 4], in_=st)

            def bn_epi(cot):
                grp, sl = cot // 2, cot % 2
                st = sts[grp][:, 2 * sl:2 * sl + 2]
                mu = pp.tile([128, 1], F32, name=f"mu{cot}")
                nc.vector.tensor_scalar(mu, st[:, 0:1], inv_n, None,
                                        op0=ALU.mult)
                m2 = pp.tile([128, 1], F32, name=f"m2{cot}")
                nc.vector.tensor_scalar(m2, st[:, 1:2], inv_n, None,
                                        op0=ALU.mult)
                musq = pp.tile([128, 1], F32, name=f"musq{cot}")
                nc.vector.tensor_tensor(out=musq, in0=mu, in1=mu, op=ALU.mult)
                var = pp.tile([128, 1], F32, name=f"var{cot}")
                nc.vector.tensor_tensor(out=var, in0=m2, in1=musq,
                                        op=ALU.subtract)
                sv = pp.tile([128, 1], F32, name=f"svq{cot}")
                nc.scalar.activation(sv, var, AF.Sqrt, bias=epst[:, :],
                                     scale=1.0)
                rstd = pp.tile([128, 1], F32, name=f"rstd{cot}")
                nc.vector.reciprocal(rstd, sv)
                scl = pp.tile([128, 1], F32, name=f"scl{cot}")
                nc.vector.tensor_tensor(out=scl, in0=gam_t[:, cot:cot + 1],
                                        in1=rstd, op=ALU.mult)
                mscl = pp.tile([128, 1], F32, name=f"mscl{cot}")
                nc.vector.tensor_tensor(out=mscl, in0=mu, in1=scl, op=ALU.mult)
                shf = pp.tile([128, 1], F32, name=f"shf{cot}")
                nc.vector.tensor_tensor(out=shf, in0=bet_t[:, cot:cot + 1],
                                        in1=mscl, op=ALU.subtract)

                for b in range(NB):
                    svec = pp.tile([128, 1], F32, name=f"sv{b}{cot}")
                    nc.vector.tensor_tensor(out=svec,
                                            in0=mask_cb[cot][:, b:b + 1],
                                            in1=scl, op=ALU.mult)
                    ot = otp.tile([128, NSP], F32, name=f"ot{b}{cot}",
                                  tag="ot", bufs=1 if sim_compat else 2)
                    if sim_compat:
                        # interp has no Prelu: max(z, 0.2 z) is identical
                        nc.vector.tensor_scalar(ot, y[b][cot], svec[:, :],
                                                shf[:, :],
                                                op0=ALU.mult, op1=ALU.add)
                        z2 = otp.tile([128, NSP], F32, name=f"z2{b}{cot}",
                                      tag="z2", bufs=1)
                        nc.vector.tensor_scalar(z2, ot, float(NEG_SLOPE),
                                                None, op0=ALU.mult)
                        nc.vector.tensor_tensor(out=ot, in0=ot, in1=z2,
                                                op=ALU.max)
                    else:
                        nc.scalar.activation(ot, y[b][cot], AF.Prelu,
                                             bias=shf[:, :],
                                             scale=svec[:, :],
                                             alpha=float(NEG_SLOPE))
                    nc.sync.dma_start(
                        out=out[b, cot * 128:(cot + 1) * 128, :, :].rearrange(
                            "p h w -> p (h w)"),
                        in_=ot)
                    if debug:
                        y32 = otp.tile([128, NSP], F32, name=f"yd{b}{cot}",
                                       tag="yd", bufs=2)
                        nc.vector.tensor_copy(y32, y[b][cot])
                        nc.sync.dma_start(out=dbg_y[b, cot, :, :], in_=y32)

            for cot in range(NCOT):
                conv_group(2, cot)
                conv_group(3, cot)
                cot_stats(cot)
                if cot % 2 == 1:
                    grp_comm(cot // 2)
                    bn_epi(cot - 1)
                    bn_epi(cot)

    import os
    if os.environ.get("NO_SPLIT_WAITS", "0") != "1":
        _split_waits(nc)
    return nc


_CACHE = {}


def kernel(x, weight, pos, neg, sal_w, sal_b, gamma, beta):
    x = np.ascontiguousarray(np.asarray(x, dtype=np.float32))
    weight = np.ascontiguousarray(np.asarray(weight, dtype=np.float32))
    sal_w = np.ascontiguousarray(np.asarray(sal_w, dtype=np.float32))
    sal_b = np.ascontiguousarray(np.asarray(sal_b, dtype=np.float32))
    gamma = np.ascontiguousarray(np.asarray(gamma, dtype=np.float32))
    beta = np.ascontiguousarray(np.asarray(beta, dtype=np.float32))
    pos_f = float(np.float32(np.asarray(pos).reshape(())))
    neg_f = float(np.float32(np.asarray(neg).reshape(())))

    r = neg_f / pos_f
    u_imm, v_imm = best_fp8_pair(r)
    s = u_imm / pos_f
    eps_imm = float(np.float32(BN_EPS) * s * s)

    import os
    debug = os.environ.get("KERNEL_DEBUG", "0") == "1"
    key = (u_imm, v_imm, eps_imm, debug)
    if key not in _CACHE:
        _CACHE[key] = build_kernel(u_imm, v_imm, eps_imm, debug)
    nc = _CACHE[key]

    in_maps = []
    for c in range(N_CORES):
        in_maps.append({
            "xs": x[c * NB:(c + 1) * NB],
            "wt": weight,
            "salw": sal_w,
            "salb": sal_b,
            "gam": gamma,
            "bet": beta,
        })
    res = run_bass_kernel_spmd(nc, in_maps, core_ids=list(range(N_CORES)))
    if debug:
        kernel.dbg = res.results
    out = np.concatenate([res.results[c]["out"] for c in range(N_CORES)], axis=0)
    return out
